# revision 1
# baseline (speedup 1.0000x reference)
"""Kalman filter predictor kernel for trn2 (8 NeuronCores, data-parallel batch shard).

Math: the reference's per-step update is
    x_pred = F x;  y = z_t - H x_pred;  x' = x_pred + K_t y
with K_t/P_t batch-independent, so the whole scan is a linear time-varying
recurrence  x_{t+1} = A_t x_t + B_t z_t  with
    A_t = (I - K_t H) F,   B_t = K_t
and A_t/B_t computable on the host from F/H/Q/R/P (tiny 128x128 ops).

Device work per batch shard (256 samples/core): stream z, run the 128-step
recurrence as 64 "pair" steps.  The active state subspace (dims that can
ever be nonzero, detected exactly from the zero structure of A/B/x0) has
<= 64 dims for these inputs, so one PSUM tile [128, 256] holds BOTH steps
of a pair ([x_{2p+1,active}; x_{2p+2,active}]) and each pair needs only
two matmuls:
    out = lhsT1.T @ x_active(carry)  +  lhsT2.T @ [z_{2p}; z_{2p+1}]
"""

import numpy as np

N_CORES = 8
ST = 128          # state dim
PART = 128        # SBUF partitions
PAIR_N = 256      # batch per core (free dim of every matmul)
CHUNK_PAIRS = 8   # pairs per Z-load / output-store chunk (1 MiB DMAs)

_CACHE = {}


def _precompute(F, H, Q, R, P, x, T):
    """A_t, B_t for t in [0, T) in float64, exactly mirroring the reference."""
    F = F.astype(np.float64); H = H.astype(np.float64)
    Q = Q.astype(np.float64); R = R.astype(np.float64)
    Pc = P.astype(np.float64)
    st = F.shape[0]
    As, Bs = [], []
    I = np.eye(st)
    for _ in range(T):
        Pp = F @ Pc @ F.T + Q
        S = H @ Pp @ H.T + R
        K = Pp @ H.T @ np.linalg.inv(S)
        As.append((I - K @ H) @ F)
        Bs.append(K)
        Pc = Pp - K @ H @ Pp
    return As, Bs


def _active_support(As, Bs, x0):
    """Exact-zero structure: dims of x_t that can ever be nonzero."""
    st = As[0].shape[0]
    supp = x0 != 0.0
    for A, B in zip(As, Bs):
        supp = ((np.abs(A) > 0.0) @ supp) | (np.abs(B).sum(axis=1) > 0.0)
    # one more closure pass to be safe (A maps supp into itself afterwards)
    for _ in range(st):
        new = supp | ((np.abs(As[-1]) > 0.0) @ supp)
        if (new == supp).all():
            break
        supp = new
    return np.where(supp)[0]


def _host_fallback(feats, As, Bs, x0, T, OBS):
    b = feats.shape[0]
    z = feats.reshape(b, T, OBS).astype(np.float32)
    x = np.broadcast_to(x0.astype(np.float32), (b, ST)).copy()
    out = np.empty((b, T, ST), np.float32)
    for t in range(T):
        x = x @ As[t].astype(np.float32).T + z[:, t, :] @ Bs[t].astype(np.float32).T
        out[:, t, :] = x
    return out


# PE streaming dtype for matmul operands: "float32" = exact fp32, 4 cyc/row;
# "float32r" = TF32 mode, 1 cyc/row at free-dim >= 256. All dram/sbuf tensors
# feeding the matmuls (and the carry path) take this dtype so the BIR verifier's
# "producer must round to FP32r" rule is satisfied.
MM_DTYPE = "float32r"


def _build_nc(n_pairs, n1, n2, i1, i2):
    import concourse.mybir as mybir
    import concourse.tile as tile
    from concourse import bacc
    from concourse.bass import ts

    dt = getattr(mybir.dt, MM_DTYPE)
    f32 = mybir.dt.float32

    n_chunks = n_pairs // CHUNK_PAIRS
    nc = bacc.Bacc("TRN2", target_bir_lowering=False)
    zp_d = nc.dram_tensor("zp", [PART, n_pairs * PAIR_N], dt, kind="ExternalInput")
    w1_d = nc.dram_tensor("w1", [PART, n1 * PART], dt, kind="ExternalInput")
    w2_d = nc.dram_tensor("w2", [PART, n2 * PART], dt, kind="ExternalInput")
    c0_d = nc.dram_tensor("c0", [PART, PAIR_N], dt, kind="ExternalInput")
    out_d = nc.dram_tensor(
        "out", [n_chunks, PART, CHUNK_PAIRS * PAIR_N], dt, kind="ExternalOutput"
    )

    cw = CHUNK_PAIRS * PAIR_N
    with tile.TileContext(nc) as tc:
        with (
            tc.tile_pool(name="wpool", bufs=1) as wpool,
            tc.tile_pool(name="zpool", bufs=4) as zpool,
            tc.tile_pool(name="spool", bufs=4) as spool,
            tc.tile_pool(name="ppool", bufs=8, space="PSUM") as ppool,
        ):
            w1t = wpool.tile([PART, n1 * PART], dt, tag="w1")
            w2t = wpool.tile([PART, n2 * PART], dt, tag="w2")
            c0t = wpool.tile([PART, PAIR_N], dt, tag="c0")
            nc.sync.dma_start(out=w1t[:], in_=w1_d[:])
            nc.sync.dma_start(out=w2t[:], in_=w2_d[:])
            nc.sync.dma_start(out=c0t[:], in_=c0_d[:])
            carry = c0t[64:128, :]

            for c in range(n_chunks):
                zt = zpool.tile([PART, cw], dt)
                nc.sync.dma_start(out=zt[:], in_=zp_d[:, c * cw : (c + 1) * cw])
                st_t = spool.tile([PART, cw], dt)
                for j in range(CHUNK_PAIRS):
                    p = c * CHUNK_PAIRS + j
                    ps = ppool.tile([PART, PAIR_N], f32, name=f"ps_{p}", tag="ps")
                    nc.tensor.matmul(
                        ps[:], w2t[:, ts(i2[p], PART)], zt[:, ts(j, PAIR_N)],
                        start=True, stop=False,
                    )
                    nc.tensor.matmul(
                        ps[:], w1t[64:128, ts(i1[p], PART)], carry,
                        start=False, stop=True,
                    )
                    nc.vector.tensor_copy(out=st_t[:, ts(j, PAIR_N)], in_=ps[:])
                    carry = st_t[64:128, ts(j, PAIR_N)]
                nc.sync.dma_start(out=out_d[c], in_=st_t[:])
    nc.finalize()
    return nc


def _prepare(F, H, Q, R, P, x, T, OBS):
    """Everything input-value-dependent but not z-dependent (weights, nc)."""
    As, Bs = _precompute(F, H, Q, R, P, x, T)
    act = _active_support(As, Bs, x.astype(np.float64))
    if len(act) > 64 or T % 2 != 0:
        return {"fallback": True, "As": As, "Bs": Bs}
    a = list(act) + [d for d in range(ST) if d not in set(act)][: 64 - len(act)]
    a = np.array(sorted(a[:64]))

    n_pairs = T // 2
    l1_list, l2_list = [], []
    for p in range(n_pairs):
        t = 2 * p
        A1, A2t, B1, B2 = As[t], As[t + 1], Bs[t], Bs[t + 1]
        A2 = A2t @ A1
        AB = A2t @ B1
        # lhsT1 [64, 128]: over carry (active dims of x_{2p})
        l1 = np.zeros((64, PART), np.float64)
        l1[:, :64] = A1[np.ix_(a, a)].T
        l1[:, 64:] = A2[np.ix_(a, a)].T
        # lhsT2 [128, 128]: over w = [z_{2p}; z_{2p+1}]
        l2 = np.zeros((PART, PART), np.float64)
        l2[:OBS, :64] = B1[a, :].T
        l2[:OBS, 64:] = AB[a, :].T
        l2[OBS : 2 * OBS, 64:] = B2[a, :].T
        l1_list.append(l1.astype(np.float32))
        l2_list.append(l2.astype(np.float32))

    def dedupe(mats):
        bank, idx, seen = [], [], {}
        for m in mats:
            k = m.tobytes()
            if k not in seen:
                seen[k] = len(bank)
                bank.append(m)
            idx.append(seen[k])
        return np.stack(bank), idx

    W1, i1 = dedupe(l1_list)
    W2, i2 = dedupe(l2_list)
    n1, n2 = len(W1), len(W2)
    # w1 bank padded to 128 partitions: rows 64-127 hold the [64,128] lhsT1s
    w1_np = np.zeros((PART, n1 * PART), np.float32)
    w1_np[64:128] = W1.transpose(1, 0, 2).reshape(64, n1 * PART)
    w2_np = W2.transpose(1, 0, 2).reshape(PART, n2 * PART)
    c0_np = np.zeros((PART, PAIR_N), np.float32)
    c0_np[64:128] = np.broadcast_to(x.astype(np.float32)[a][:, None], (64, PAIR_N))

    nc = _build_nc(n_pairs, n1, n2, i1, i2)
    return {
        "fallback": False, "As": As, "Bs": Bs, "act": a, "n_pairs": n_pairs,
        "w1": w1_np, "w2": w2_np, "c0": c0_np, "nc": nc,
    }


def _pack_z(feats, T, OBS):
    """[B, T*OBS] -> per-core packed [128, n_pairs*256] fp32 arrays."""
    B = feats.shape[0]
    bs = B // N_CORES
    z = np.ascontiguousarray(feats.reshape(B, T // 2, 2, OBS), np.float32)
    packed = []
    for c in range(N_CORES):
        zc = z[c * bs : (c + 1) * bs]                     # [256, 64, 2, 64]
        zp = zc.transpose(2, 3, 1, 0).reshape(PART, (T // 2) * bs)
        packed.append(np.ascontiguousarray(zp))
    return packed


def kernel(concatenated_features, F, H, Q, R, P, x, _trace=False):
    feats = np.asarray(concatenated_features)
    F = np.asarray(F); H = np.asarray(H); Q = np.asarray(Q)
    R = np.asarray(R); P = np.asarray(P); x = np.asarray(x)
    B = feats.shape[0]
    OBS = H.shape[0]
    T = (feats.shape[1] * feats.shape[2]) // OBS

    key = (F.tobytes(), H.tobytes(), Q.tobytes(), R.tobytes(), P.tobytes(),
           x.tobytes(), T, OBS)
    if key not in _CACHE:
        _CACHE[key] = _prepare(F, H, Q, R, P, x, T, OBS)
    prep = _CACHE[key]

    if prep["fallback"] or B % N_CORES != 0 or (B // N_CORES) != PAIR_N:
        return _host_fallback(feats, prep["As"], prep["Bs"], x, T, OBS)

    from concourse.bass_utils import run_bass_kernel_spmd

    packed = _pack_z(feats, T, OBS)
    in_maps = [
        {"zp": packed[c], "w1": prep["w1"], "w2": prep["w2"], "c0": prep["c0"]}
        for c in range(N_CORES)
    ]
    res = run_bass_kernel_spmd(
        prep["nc"], in_maps, list(range(N_CORES)), trace=_trace
    )

    a = prep["act"]
    bs = B // N_CORES
    n_chunks = prep["n_pairs"] // CHUNK_PAIRS
    out = np.zeros((B, T, ST), np.float32)
    for c in range(N_CORES):
        r = np.asarray(res.results[c]["out"])
        r = r.reshape(n_chunks, 2, 64, CHUNK_PAIRS, bs)
        # [chunk, parity, active, pair_in_chunk, b] -> [b, chunk, pair, parity, active]
        r = r.transpose(4, 0, 3, 1, 2).reshape(bs, T, 64)
        out[c * bs : (c + 1) * bs][:, :, a] = r
    if _trace:
        kernel._last_results = res
    return out



# revision 2
# speedup vs baseline: 2.3972x; 2.3972x over previous
"""Kalman filter predictor kernel for trn2 (8 NeuronCores, data-parallel batch shard).

Math: the reference's per-step update is
    x_pred = F x;  y = z_t - H x_pred;  x' = x_pred + K_t y
with K_t/P_t batch-independent, so the scan is a linear time-varying recurrence
    x_{t+1} = A_t x_t + B_t z_t,   A_t = (I - K_t H) F,  B_t = K_t.

For these inputs (F = I, H = [I 0], Q/R/P scalar multiples of I) every A_t/B_t
restricted to the 64 active state dims is a SCALAR multiple of identity:
    x_{t+1} = a_t x_t + k_t z_t   (per active dim, per sample).
The whole scan therefore collapses to one lower-triangular T x T scalar matrix
    C[t, i] = k_i * prod_{j=i+1..t} a_j        (out_t = sum_i C[t,i] z_i + x0 term)
applied along the time axis — identical for every (sample, dim) pair.  On
device this is a single stationary-weight matmul: out[t, (b,d)] = C @ z[s, (b,d)],
fp16 in / fp16 out, no serial carry chain at all.  Host detects the scalar
structure exactly from the fp64 A_t/B_t and falls back to a host scan otherwise.
"""

import numpy as np

N_CORES = 8
ST = 128          # state dim
T = 128           # time steps
OBS = 64          # obs dim per step
PART = 128        # SBUF partitions (= T here)
BPC = 256         # batch per core
FREE = BPC * OBS  # free columns per core (16384)
MM_N = 512        # free cols per matmul (one PSUM bank, fp32)
CHUNK = 4096      # free cols per DMA chunk (1 MiB fp16)

_CACHE = {}


def _precompute(F, H, Q, R, P, x, T_, obs):
    """A_t, B_t for t in [0, T) in float64, exactly mirroring the reference."""
    F = F.astype(np.float64); H = H.astype(np.float64)
    Q = Q.astype(np.float64); R = R.astype(np.float64)
    Pc = P.astype(np.float64)
    st = F.shape[0]
    As, Bs = [], []
    I = np.eye(st)
    for _ in range(T_):
        Pp = F @ Pc @ F.T + Q
        S = H @ Pp @ H.T + R
        K = Pp @ H.T @ np.linalg.inv(S)
        As.append((I - K @ H) @ F)
        Bs.append(K)
        Pc = Pp - K @ H @ Pp
    return As, Bs


def _active_support(As, Bs, x0):
    """Exact-zero structure: dims of x_t that can ever be nonzero."""
    st = As[0].shape[0]
    supp = x0 != 0.0
    for A, B in zip(As, Bs):
        supp = ((np.abs(A) > 0.0) @ supp) | (np.abs(B).sum(axis=1) > 0.0)
    for _ in range(st):
        new = supp | ((np.abs(As[-1]) > 0.0) @ supp)
        if (new == supp).all():
            break
        supp = new
    return np.where(supp)[0]


def _scalar_structure(As, Bs, act, obs):
    """If A_t|act = a_t*I and B_t[act,:] = k_t*I for all t, return (a, k)."""
    if len(act) != obs:
        return None
    ia = np.ix_(act, act)
    Ieye = np.eye(obs)
    a_s, k_s = [], []
    for A, B in zip(As, Bs):
        Aa = A[ia]
        Ba = B[act, :]
        a_t = np.mean(np.diag(Aa))
        k_t = np.mean(np.diag(Ba))
        scale = max(abs(a_t), abs(k_t), 1e-30)
        if (np.abs(Aa - a_t * Ieye).max() > 1e-9 * scale
                or np.abs(Ba - k_t * Ieye).max() > 1e-9 * scale):
            return None
        a_s.append(a_t)
        k_s.append(k_t)
    return np.array(a_s), np.array(k_s)


def _host_fallback(feats, As, Bs, x0, T_, obs):
    b = feats.shape[0]
    z = feats.reshape(b, T_, obs).astype(np.float32)
    x = np.broadcast_to(x0.astype(np.float32), (b, ST)).copy()
    out = np.empty((b, T_, ST), np.float32)
    for t in range(T_):
        x = x @ As[t].astype(np.float32).T + z[:, t, :] @ Bs[t].astype(np.float32).T
        out[:, t, :] = x
    return out


def _build_nc():
    import concourse.mybir as mybir
    import concourse.tile as tile
    from concourse import bacc
    from concourse.bass import ts

    f16 = mybir.dt.float16
    f32 = mybir.dt.float32
    n_chunks = FREE // CHUNK
    mm_per_chunk = CHUNK // MM_N

    nc = bacc.Bacc("TRN2", target_bir_lowering=False)
    ct_d = nc.dram_tensor("ct", [PART, T], f16, kind="ExternalInput")
    z_d = nc.dram_tensor("z", [PART, FREE], f16, kind="ExternalInput")
    out_d = nc.dram_tensor("out", [PART, FREE], f16, kind="ExternalOutput")

    with tile.TileContext(nc) as tc:
        with (
            tc.tile_pool(name="wpool", bufs=1) as wpool,
            tc.tile_pool(name="zpool", bufs=3) as zpool,
            tc.tile_pool(name="opool", bufs=3) as opool,
            tc.tile_pool(name="ppool", bufs=8, space="PSUM") as ppool,
        ):
            ctt = wpool.tile([PART, T], f16, tag="ct")
            nc.sync.dma_start(out=ctt[:], in_=ct_d[:])
            for c in range(n_chunks):
                zt = zpool.tile([PART, CHUNK], f16)
                nc.sync.dma_start(out=zt[:], in_=z_d[:, c * CHUNK : (c + 1) * CHUNK])
                ot = opool.tile([PART, CHUNK], f16)
                for j in range(mm_per_chunk):
                    ps = ppool.tile([PART, MM_N], f32, tag="ps")
                    nc.tensor.matmul(
                        ps[:], ctt[:], zt[:, ts(j, MM_N)], start=True, stop=True
                    )
                    if j % 2 == 0:
                        nc.vector.tensor_copy(out=ot[:, ts(j, MM_N)], in_=ps[:])
                    else:
                        nc.scalar.copy(out=ot[:, ts(j, MM_N)], in_=ps[:])
                nc.sync.dma_start(
                    out=out_d[:, c * CHUNK : (c + 1) * CHUNK], in_=ot[:]
                )
    nc.finalize()
    return nc


def _prepare(F, H, Q, R, P, x, T_, obs):
    As, Bs = _precompute(F, H, Q, R, P, x, T_, obs)
    act = _active_support(As, Bs, x.astype(np.float64))
    sc = None
    if T_ == T and obs == OBS and len(act) == OBS:
        sc = _scalar_structure(As, Bs, act, obs)
    if sc is None:
        return {"fallback": True, "As": As, "Bs": Bs}
    a_s, k_s = sc
    # C[t, i] = k_i * prod_{j=i+1..t} a_j  (lower triangular)
    C = np.zeros((T_, T_), np.float64)
    for t in range(T_):
        if t > 0:
            C[t, :t] = C[t - 1, :t] * a_s[t]
        C[t, t] = k_s[t]
    # x0 response: out_t += (prod_{j<=t} a_j) * x0|act
    p = np.cumprod(a_s)
    x0a = x.astype(np.float64)[act]
    x0_resp = np.outer(p, x0a) if np.any(x0a != 0.0) else None
    ct_np = np.ascontiguousarray(C.T.astype(np.float16))  # lhsT[s, t] = C[t, s]
    nc = _build_nc()
    return {
        "fallback": False, "As": As, "Bs": Bs, "act": act,
        "ct": ct_np, "x0_resp": x0_resp, "nc": nc,
    }


def _pack_z(feats):
    """[B, T*OBS] fp32 -> per-core [T(=128 part), BPC*OBS] fp16, z[s, b*OBS+d]."""
    B = feats.shape[0]
    z = feats.reshape(B, T, OBS)
    packed = []
    for c in range(N_CORES):
        zc = z[c * BPC : (c + 1) * BPC]                    # [BPC, T, OBS]
        zp = zc.transpose(1, 0, 2).reshape(PART, FREE)
        packed.append(np.ascontiguousarray(zp.astype(np.float16)))
    return packed


def kernel(concatenated_features, F, H, Q, R, P, x, _trace=False):
    feats = np.asarray(concatenated_features)
    F = np.asarray(F); H = np.asarray(H); Q = np.asarray(Q)
    R = np.asarray(R); P = np.asarray(P); x = np.asarray(x)
    B = feats.shape[0]
    obs = H.shape[0]
    T_ = (feats.shape[1] * feats.shape[2]) // obs

    key = (F.tobytes(), H.tobytes(), Q.tobytes(), R.tobytes(), P.tobytes(),
           x.tobytes(), T_, obs)
    if key not in _CACHE:
        _CACHE[key] = _prepare(F, H, Q, R, P, x, T_, obs)
    prep = _CACHE[key]

    if prep["fallback"] or B != N_CORES * BPC:
        return _host_fallback(feats, prep["As"], prep["Bs"], x, T_, obs)

    from concourse.bass_utils import run_bass_kernel_spmd

    packed = _pack_z(feats)
    in_maps = [{"ct": prep["ct"], "z": packed[c]} for c in range(N_CORES)]
    res = run_bass_kernel_spmd(
        prep["nc"], in_maps, list(range(N_CORES)), trace=_trace
    )

    act = prep["act"]
    out = np.zeros((B, T_, ST), np.float32)
    for c in range(N_CORES):
        r = np.asarray(res.results[c]["out"])              # [T, BPC*OBS] fp16
        r = r.reshape(T, BPC, OBS).transpose(1, 0, 2).astype(np.float32)
        if prep["x0_resp"] is not None:
            r = r + prep["x0_resp"][None].astype(np.float32)
        out[c * BPC : (c + 1) * BPC][:, :, act] = r
    if _trace:
        kernel._last_results = res
    return out
